# revision 2
# baseline (speedup 1.0000x reference)
"""CoPEGate Trainium2 kernel.

Computes out[b,h,t,s] = sigmoid((Q K^T)[b,h,t,s] / sqrt(D)) * (P P^T)[t,s] / sqrt(D)
for B=2, H=12, T=2048, D=64 (fp32 in/out), distributed over 8 NeuronCores.

Sharding: the 24 (b,h) pairs are split 3-per-core (head-parallel); the
positional matrix P is replicated and its T x T bias is computed on every
core (reused across that core's 3 heads). No cross-device communication.

The big lever vs the fp32 baseline: the harness tolerance is rel-err 2e-2
(L2), so the output is written to HBM as fp16 (adds ~3e-4 L2 rounding) and
upcast to fp32 on the host. That halves output DMA from 48 MiB to 24 MiB
per core and moves the bottleneck from HBM writes (~147 us floor) to the
elementwise engines:
  ACT  48 sigmoid stripes x ~1850 ns          ~= 89 us busy  <- bound
  DVE  48 fp16 muls (2x mode) + 32 half-copies ~= 92 us busy
  DMA  ~27 MiB wire                            ~= 78 us
Per-core dataflow (per row-tile of 128 rows, 16 tiles):
  pos stripe: PE matmul (f32r, scale pre-folded on host)      -> PSUM
              2x half-width DVE tensor_copy -> SBUF fp16
  per head:   PE matmul qT.T @ kT (fp16 operands)             -> PSUM
              ACT Sigmoid(scores/sqrt(D))   -> SBUF fp16 gate
              DVE tensor_mul(gate, pos)     -> SBUF fp16 (2x_1p mode)
              one 512 KiB contiguous DMA per stripe           -> HBM
PSUM holds exactly two [128,2048] f32 stripes (8 banks); the pos copy is
split into two half-width copies so the freed banks can be refilled by the
next score matmul before the full copy retires (subtile deps), removing
the buffer-rotation bubble that a one-shot copy would put in front of the
next sigmoid.

PE utilization: the K=64 contraction only uses half the 128-row PE array,
so operands sit in alternating partition halves (pos & head0 at partitions
0-63, heads 1 & 2 at 64-127) and stripes issue in the order pos, h1, h0,
h2 -- adjacent stripes' matmuls hit disjoint PE row groups and overlap.

Precision: q/k host-rounded to fp16 (10 mantissa bits); pos matmul in
float32r; output stripes fp16. End-to-end rel err ~3.5e-4 vs the 2e-2
gate.
"""

import math
import os
import sys

import numpy as np

sys.path.insert(0, "/opt/trn_rl_repo")

B, H, T, D = 2, 12, 2048, 64
N_CORES = 8
HPC = (B * H) // N_CORES  # heads per core
PT = 128  # output row-tile height (SBUF/PSUM partitions)
NT = T // PT  # row tiles
NCHUNK = 512  # matmul moving-operand free dim (one PSUM bank of fp32)
NCH = T // NCHUNK
INV_SQRT_D = 1.0 / math.sqrt(D)

_NC_CACHE = {}


def _build_nc():
    import concourse.bass as bass
    from concourse import bacc, mybir, tile

    f32 = mybir.dt.float32
    f32r = mybir.dt.float32r
    f16 = mybir.dt.float16
    Sigmoid = mybir.ActivationFunctionType.Sigmoid
    Copy = mybir.ActivationFunctionType.Copy

    nc = bacc.Bacc("TRN2", target_bir_lowering=False)

    qT = nc.dram_tensor("qT", [HPC, D, T], f16, kind="ExternalInput")
    kT = nc.dram_tensor("kT", [HPC, D, T], f16, kind="ExternalInput")
    # pos scale 1/sqrt(D) is folded into pT on the host (split across both
    # factors), so the pos matmul lands pre-scaled in PSUM.
    pT = nc.dram_tensor("pT", [D, T], f32r, kind="ExternalInput")
    out = nc.dram_tensor("out", [HPC, T, T], f16, kind="ExternalOutput")

    with tile.TileContext(nc) as tc:
        with tc.tile_pool(name="ins", bufs=1) as ins_pool, \
             tc.tile_pool(name="pos", bufs=3) as pos_pool, \
             tc.tile_pool(name="gate", bufs=6) as gate_pool, \
             tc.tile_pool(name="outs", bufs=12) as outs_pool, \
             tc.tile_pool(name="ps", bufs=2, space="PSUM") as ps_pool:

            # Inputs live as 512-wide column-chunk tiles so the first
            # matmul only waits for ~0.3 MiB of input DMA (separate tiles
            # => separate scheduler dependencies). Heads 0+1 share
            # [128, 512] tiles (head 1 on partitions 64-127); head 2
            # occupies the upper half of its own tiles so its matmuls use
            # the upper PE row group.
            k01_c, q01_c, p_c, k2_c, q2_c = [], [], [], [], []
            qT01 = qT[0:2].rearrange("h d t -> (h d) t")
            kT01 = kT[0:2].rearrange("h d t -> (h d) t")
            for j in range(NCH):
                jsl = bass.ts(j, NCHUNK)
                kc = ins_pool.tile([2 * D, NCHUNK], f16, tag=f"k01_{j}")
                nc.sync.dma_start(out=kc, in_=kT01[:, jsl])
                k01_c.append(kc)
                qc = ins_pool.tile([2 * D, NCHUNK], f16, tag=f"q01_{j}")
                nc.sync.dma_start(out=qc, in_=qT01[:, jsl])
                q01_c.append(qc)
                pc = ins_pool.tile([D, NCHUNK], f32r, tag=f"p_{j}")
                nc.sync.dma_start(out=pc, in_=pT[:, jsl])
                p_c.append(pc)
            for j in range(NCH):
                jsl = bass.ts(j, NCHUNK)
                kc = ins_pool.tile([2 * D, NCHUNK], f16, tag=f"k2_{j}")
                nc.sync.dma_start(out=kc[D : 2 * D, :], in_=kT[2][:, jsl])
                k2_c.append(kc)
                qc = ins_pool.tile([2 * D, NCHUNK], f16, tag=f"q2_{j}")
                nc.sync.dma_start(out=qc[D : 2 * D, :], in_=qT[2][:, jsl])
                q2_c.append(qc)

            def q_lhsT(h, it):
                # lhsT [64, 128] = q chunk tile (it//4), 128-col slice.
                sl = bass.ts(it % (NCHUNK // PT), PT)
                if h == 2:
                    return q2_c[it // (NCHUNK // PT)][D : 2 * D, sl]
                lo, hi = (0, D) if h == 0 else (D, 2 * D)
                return q01_c[it // (NCHUNK // PT)][lo:hi, sl]

            def p_lhsT(it):
                sl = bass.ts(it % (NCHUNK // PT), PT)
                return p_c[it // (NCHUNK // PT)][:, sl]

            def rhs(h, j):
                if h is None:
                    return p_c[j][:, :]
                if h == 2:
                    return k2_c[j][D : 2 * D, :]
                lo, hi = (0, D) if h == 0 else (D, 2 * D)
                return k01_c[j][lo:hi, :]

            def mm_stripe(psum, h, it):
                for j in range(NCH):
                    nc.tensor.matmul(
                        psum[:, bass.ts(j, NCHUNK)],
                        p_lhsT(it) if h is None else q_lhsT(h, it),
                        rhs(h, j),
                        start=True,
                        stop=True,
                    )

            def post(h, sp, pos_sb, tsl):
                gate = gate_pool.tile([PT, T], f16, tag="gate")
                o = outs_pool.tile([PT, T], f16, tag="o")
                nc.scalar.activation(gate, sp, Sigmoid, scale=INV_SQRT_D)
                nc.vector.tensor_mul(o, gate, pos_sb)
                nc.sync.dma_start(out=out[h, tsl, :], in_=o)

            # --- tile 0: chunk-major software pipeline -----------------
            # Emitted in dataflow order, chunk by chunk, so every engine's
            # FIFO sees tile 0's chunk c before chunk c+1 work and the
            # first output bytes reach HBM as early as possible. Head 1
            # leads because its q/k chunks are DMA'd first. Tile 0's pos
            # copies run on ACT (it is otherwise idle during the input
            # ramp); all later pos copies run on DVE.
            tsl0 = bass.ts(0, PT)
            sp1 = ps_pool.tile([PT, T], f32, tag="ps")
            pp0 = ps_pool.tile([PT, T], f32, tag="ps")
            pos0 = pos_pool.tile([PT, T], f16, tag="pos")
            gate0 = gate_pool.tile([PT, T], f16, tag="gate")
            o0 = outs_pool.tile([PT, T], f16, tag="o")
            for c in range(NCH):
                csl = bass.ts(c, NCHUNK)
                nc.tensor.matmul(
                    sp1[:, csl], q_lhsT(1, 0), rhs(1, c), start=True, stop=True
                )
                nc.tensor.matmul(
                    pp0[:, csl], p_lhsT(0), rhs(None, c), start=True, stop=True
                )
                nc.scalar.activation(
                    gate0[:, csl], sp1[:, csl], Sigmoid, scale=INV_SQRT_D
                )
                nc.scalar.activation(pos0[:, csl], pp0[:, csl], Copy)
                nc.vector.tensor_mul(o0[:, csl], gate0[:, csl], pos0[:, csl])
                nc.sync.dma_start(out=out[1, tsl0, csl], in_=o0[:, csl])
            for h in (0, 2):
                sp = ps_pool.tile([PT, T], f32, tag="ps")
                mm_stripe(sp, h, 0)
                post(h, sp, pos0, tsl0)

            # --- steady-state tiles ------------------------------------
            # Stripe order pos, h1, h0, h2: PSUM rotation A=pos/h0,
            # B=h1/h2. The pos copy is emitted as two half-width DVE
            # copies so h0's chunk matmuls can refill the A banks behind
            # the copy front (subtile deps) instead of waiting for a
            # one-shot copy to retire.
            HW = T // 2
            for it in range(1, NT):
                tsl = bass.ts(it, PT)
                pp = ps_pool.tile([PT, T], f32, tag="ps")
                mm_stripe(pp, None, it)
                pos_sb = pos_pool.tile([PT, T], f16, tag="pos")
                for half in range(2):
                    hsl = bass.ts(half, HW)
                    nc.vector.tensor_copy(pos_sb[:, hsl], pp[:, hsl])
                for h in (1, 0, 2):
                    sp = ps_pool.tile([PT, T], f32, tag="ps")
                    mm_stripe(sp, h, it)
                    post(h, sp, pos_sb, tsl)

    nc.finalize()
    return nc


def _get_nc():
    if "nc" not in _NC_CACHE:
        _NC_CACHE["nc"] = _build_nc()
    return _NC_CACHE["nc"]


def kernel(query, key, pos_embed_weight):
    query = np.asarray(query, dtype=np.float32)
    key = np.asarray(key, dtype=np.float32)
    pos_embed_weight = np.asarray(pos_embed_weight, dtype=np.float32)

    q = query.reshape(B * H, T, D)
    k = key.reshape(B * H, T, D)
    # Fold the pos-bias 1/sqrt(D) into the (replicated) P operand: the
    # matmul computes (s*P)(s*P)^T = P P^T / sqrt(D) with s = D**-0.25.
    p_t = np.ascontiguousarray(
        pos_embed_weight[:T].T * np.float32(D**-0.25)
    )  # [D, T]

    in_maps = []
    for c in range(N_CORES):
        hs = slice(c * HPC, (c + 1) * HPC)
        in_maps.append(
            {
                "qT": np.ascontiguousarray(
                    q[hs].transpose(0, 2, 1).astype(np.float16)
                ),
                "kT": np.ascontiguousarray(
                    k[hs].transpose(0, 2, 1).astype(np.float16)
                ),
                "pT": p_t,
            }
        )

    from concourse.bass_utils import run_bass_kernel_spmd

    nc = _get_nc()
    try:
        res = run_bass_kernel_spmd(
            nc,
            in_maps,
            core_ids=list(range(N_CORES)),
            trace=bool(os.environ.get("KERNEL_TRACE")),
        )
    except Exception:
        # One retry for transient runtime/compile hiccups.
        res = run_bass_kernel_spmd(
            nc, in_maps, core_ids=list(range(N_CORES)), trace=False
        )
    kernel.last_results = res

    full = np.empty((B * H, T, T), dtype=np.float32)
    for c in range(N_CORES):
        full[c * HPC : (c + 1) * HPC] = res.results[c]["out"]
    return full.reshape(B, H, T, T)


kernel.last_results = None


# revision 5
# speedup vs baseline: 1.0790x; 1.0790x over previous
"""CoPEGate Trainium2 kernel.

Computes out[b,h,t,s] = sigmoid((Q K^T)[b,h,t,s] / sqrt(D)) * (P P^T)[t,s] / sqrt(D)
for B=2, H=12, T=2048, D=64 (fp32 in/out), distributed over 8 NeuronCores.

Sharding: the 24 (b,h) pairs are split 3-per-core (head-parallel); the
positional matrix P is replicated and its T x T bias is computed on every
core (reused across that core's 3 heads). No cross-device communication.

The big lever vs the fp32 baseline: the harness tolerance is rel-err 2e-2
(L2), so the output is written to HBM as fp16 (adds ~3e-4 L2 rounding) and
upcast to fp32 on the host. That halves output DMA from 48 MiB to 24 MiB
per core and moves the bottleneck from HBM writes (~147 us floor) to the
elementwise engines + PE:
  ACT  96 half-stripe sigmoids x ~997 ns      ~= 96 us busy  <- bound
  DVE  fp16 muls (2x mode) + pos casts        ~= 96 us busy
  PE   256 chunk matmuls @ 427 ns (1.2 GHz
       mid p-state; 2.4 GHz after 3 us busy)  ~= 85-110 us
  DMA  ~27 MiB wire                           ~= 78 us

Pipeline structure (per row-tile of 128 rows, 16 tiles): all PSUM stripes
are HALF-width [128, 1024] (2 banks), 4 buffers = all 8 banks. A
half-stripe's matmul->sigmoid round trip is ~2.1 us with 4 in flight, so
PSUM recycling never gates the ACT engine (a full-width 2-buffer rotation
measured 11.3 us/tile on HW because each stripe's fill->consume->refill
loop serialized, and the pos stripe's DVE casts gated the next score
stripe's matmuls).

The pos stripe for tile it+1 is computed DURING tile it (matmul + two
half-width DVE casts ordered between the muls by dependency readiness), so
pos never sits in the critical path. PSUM ring (4 slots, alloc order):
  s1a->P2, s1b->P3, s0a->P0, s0b->P1, pp_a'->P2, s2a->P3, s2b->P0, pp_b'->P1
which gives every sigmoid >= 850 ns of fill slack at steady state.

PE utilization: the K=64 contraction only uses half the 128-row PE array,
so operands sit in alternating partition halves (pos & head0 at partitions
0-63, heads 1 & 2 at 64-127); adjacent stripes' matmuls hit disjoint PE
row groups and overlap at the boundaries.

Precision: q/k host-rounded to fp16; pos matmul fp16 (pre-scaled by
D**-0.25 on the host so PSUM holds the final pos bias); output fp16.
End-to-end rel err ~4e-4 vs the 2e-2 gate.
"""

import math
import os
import sys

import numpy as np

sys.path.insert(0, "/opt/trn_rl_repo")

B, H, T, D = 2, 12, 2048, 64
N_CORES = 8
HPC = (B * H) // N_CORES  # heads per core
PT = 128  # output row-tile height (SBUF/PSUM partitions)
NT = T // PT  # row tiles
NCHUNK = 512  # matmul moving-operand free dim (one PSUM bank of fp32)
NCH = T // NCHUNK
HW = T // 2  # half-stripe width: [128, HW] f32 = 2 PSUM banks
INV_SQRT_D = 1.0 / math.sqrt(D)

# Offload head 2's multiply to GPSIMD to keep DVE under the ACT-bound tile
# period (DVE otherwise carries 2 casts + 6 half muls ~= 6.3 us/tile).
GPSIMD_MUL2 = True

_NC_CACHE = {}


def _build_nc():
    import concourse.bass as bass
    from concourse import bacc, mybir, tile

    f32 = mybir.dt.float32
    f16 = mybir.dt.float16
    Sigmoid = mybir.ActivationFunctionType.Sigmoid

    nc = bacc.Bacc("TRN2", target_bir_lowering=False)

    qT = nc.dram_tensor("qT", [HPC, D, T], f16, kind="ExternalInput")
    kT = nc.dram_tensor("kT", [HPC, D, T], f16, kind="ExternalInput")
    # pos scale 1/sqrt(D) is folded into pT on the host (split across both
    # factors), so the pos matmul lands pre-scaled in PSUM.
    pT = nc.dram_tensor("pT", [D, T], f16, kind="ExternalInput")
    out = nc.dram_tensor("out", [HPC, T, T], f16, kind="ExternalOutput")

    with tile.TileContext(nc) as tc:
        with tc.tile_pool(name="ins", bufs=1) as ins_pool, \
             tc.tile_pool(name="pos", bufs=3) as pos_pool, \
             tc.tile_pool(name="gate", bufs=6) as gate_pool, \
             tc.tile_pool(name="outs", bufs=12) as outs_pool, \
             tc.tile_pool(name="ps", bufs=4, space="PSUM") as ps_pool:

            # Inputs live as 512-wide column-chunk tiles so the first
            # matmul only waits for ~0.4 MiB of input DMA (separate tiles
            # => separate scheduler dependencies). Heads 0+1 share
            # [128, 512] tiles (head 1 on partitions 64-127); head 2
            # occupies the upper half of its own tiles so its matmuls use
            # the upper PE row group. DMA order interleaves k01/q01/p so
            # the score pipeline and the pos pipeline both start early.
            k01_c, q01_c, p_c, k2_c, q2_c = [], [], [], [], []
            qT01 = qT[0:2].rearrange("h d t -> (h d) t")
            kT01 = kT[0:2].rearrange("h d t -> (h d) t")
            for j in range(NCH):
                jsl = bass.ts(j, NCHUNK)
                kc = ins_pool.tile([2 * D, NCHUNK], f16, tag=f"k01_{j}")
                nc.sync.dma_start(out=kc, in_=kT01[:, jsl])
                k01_c.append(kc)
                qc = ins_pool.tile([2 * D, NCHUNK], f16, tag=f"q01_{j}")
                nc.sync.dma_start(out=qc, in_=qT01[:, jsl])
                q01_c.append(qc)
                pc = ins_pool.tile([D, NCHUNK], f16, tag=f"p_{j}")
                nc.sync.dma_start(out=pc, in_=pT[:, jsl])
                p_c.append(pc)
            for j in range(NCH):
                jsl = bass.ts(j, NCHUNK)
                kc = ins_pool.tile([2 * D, NCHUNK], f16, tag=f"k2_{j}")
                nc.sync.dma_start(out=kc[D : 2 * D, :], in_=kT[2][:, jsl])
                k2_c.append(kc)
                qc = ins_pool.tile([2 * D, NCHUNK], f16, tag=f"q2_{j}")
                nc.sync.dma_start(out=qc[D : 2 * D, :], in_=qT[2][:, jsl])
                q2_c.append(qc)

            def q_lhsT(h, it):
                # lhsT [64, 128] = q chunk tile (it//4), 128-col slice.
                sl = bass.ts(it % (NCHUNK // PT), PT)
                if h == 2:
                    return q2_c[it // (NCHUNK // PT)][D : 2 * D, sl]
                lo, hi = (0, D) if h == 0 else (D, 2 * D)
                return q01_c[it // (NCHUNK // PT)][lo:hi, sl]

            def p_lhsT(it):
                sl = bass.ts(it % (NCHUNK // PT), PT)
                return p_c[it // (NCHUNK // PT)][:, sl]

            def rhs(h, j):
                if h is None:
                    return p_c[j][:, :]
                if h == 2:
                    return k2_c[j][D : 2 * D, :]
                lo, hi = (0, D) if h == 0 else (D, 2 * D)
                return k01_c[j][lo:hi, :]

            def mm_half(psum, h, it, half):
                # Fill one [128, HW] half-stripe = 2 one-bank matmuls.
                for jj in range(2):
                    j = 2 * half + jj
                    nc.tensor.matmul(
                        psum[:, bass.ts(jj, NCHUNK)],
                        p_lhsT(it) if h is None else q_lhsT(h, it),
                        rhs(h, j),
                        start=True,
                        stop=True,
                    )

            def pos_half(pos_sb, it, half):
                # pos half-stripe for tile `it`: matmul + DVE cast f32->f16.
                pp = ps_pool.tile([PT, HW], f32, tag="ps")
                mm_half(pp, None, it, half)
                nc.vector.tensor_copy(pos_sb[:, bass.ts(half, HW)], pp)

            def head_sig(h, it):
                # Score half-stripes + half-width sigmoids -> full gate.
                gate = gate_pool.tile([PT, T], f16, tag="gate")
                for half in range(2):
                    sp = ps_pool.tile([PT, HW], f32, tag="ps")
                    mm_half(sp, h, it, half)
                    nc.scalar.activation(
                        gate[:, bass.ts(half, HW)], sp, Sigmoid, scale=INV_SQRT_D
                    )
                return gate

            def head_mul_dma(h, it, gate, pos_sb, eng):
                o = outs_pool.tile([PT, T], f16, tag="o")
                if eng is nc.gpsimd:
                    eng.tensor_mul(o, gate, pos_sb)
                else:
                    for half in range(2):
                        hsl = bass.ts(half, HW)
                        eng.tensor_mul(o[:, hsl], gate[:, hsl], pos_sb[:, hsl])
                nc.sync.dma_start(out=out[h, bass.ts(it, PT), :], in_=o)

            # --- pos prologue for tile 0 -------------------------------
            pos_cur = pos_pool.tile([PT, T], f16, tag="pos")
            for half in range(2):
                pos_half(pos_cur, 0, half)

            # --- tiles -------------------------------------------------
            # Emission order per tile (engine program order):
            #   PE : s1a s1b s0a s0b pp_a' s2a s2b pp_b'
            #   ACT: sig1a sig1b sig0a sig0b sig2a sig2b
            #   DVE: mul1a mul1b cast_a' mul0a mul0b cast_b' (mul2 halves)
            # pos for tile it+1 is produced between this tile's muls.
            for it in range(NT):
                gate1 = head_sig(1, it)
                head_mul_dma(1, it, gate1, pos_cur, nc.vector)

                gate0 = head_sig(0, it)
                pos_next = None
                if it + 1 < NT:
                    pos_next = pos_pool.tile([PT, T], f16, tag="pos")
                    pos_half(pos_next, it + 1, 0)
                head_mul_dma(0, it, gate0, pos_cur, nc.vector)

                gate2 = head_sig(2, it)
                if pos_next is not None:
                    pos_half(pos_next, it + 1, 1)
                head_mul_dma(
                    2, it, gate2, pos_cur,
                    nc.gpsimd if GPSIMD_MUL2 else nc.vector,
                )
                if pos_next is not None:
                    pos_cur = pos_next

    nc.finalize()
    return nc


def _get_nc():
    if "nc" not in _NC_CACHE:
        _NC_CACHE["nc"] = _build_nc()
    return _NC_CACHE["nc"]


def kernel(query, key, pos_embed_weight):
    query = np.asarray(query, dtype=np.float32)
    key = np.asarray(key, dtype=np.float32)
    pos_embed_weight = np.asarray(pos_embed_weight, dtype=np.float32)

    q = query.reshape(B * H, T, D)
    k = key.reshape(B * H, T, D)
    # Fold the pos-bias 1/sqrt(D) into the (replicated) P operand: the
    # matmul computes (s*P)(s*P)^T = P P^T / sqrt(D) with s = D**-0.25.
    p_t = np.ascontiguousarray(
        (pos_embed_weight[:T].T * np.float32(D**-0.25)).astype(np.float16)
    )  # [D, T]

    in_maps = []
    for c in range(N_CORES):
        hs = slice(c * HPC, (c + 1) * HPC)
        in_maps.append(
            {
                "qT": np.ascontiguousarray(
                    q[hs].transpose(0, 2, 1).astype(np.float16)
                ),
                "kT": np.ascontiguousarray(
                    k[hs].transpose(0, 2, 1).astype(np.float16)
                ),
                "pT": p_t,
            }
        )

    from concourse.bass_utils import run_bass_kernel_spmd

    nc = _get_nc()
    try:
        res = run_bass_kernel_spmd(
            nc,
            in_maps,
            core_ids=list(range(N_CORES)),
            trace=bool(os.environ.get("KERNEL_TRACE")),
        )
    except Exception:
        # One retry for transient runtime/compile hiccups.
        res = run_bass_kernel_spmd(
            nc, in_maps, core_ids=list(range(N_CORES)), trace=False
        )
    kernel.last_results = res

    full = np.empty((B * H, T, T), dtype=np.float32)
    for c in range(N_CORES):
        full[c * HPC : (c + 1) * HPC] = res.results[c]["out"]
    return full.reshape(B, H, T, T)


kernel.last_results = None
